# revision 12
# baseline (speedup 1.0000x reference)
"""Trainium2 Bass kernel for a transformer decoder layer (self-attn + cross-attn + FFN).

Sharding (8 cores): sequence-parallel, no collectives. Cores 0-3 handle batch 0,
cores 4-7 batch 1. The core with group-rank r owns query rows r::4 of its batch
(interleaved, so the causal workload is identical on every core). Every core
computes K/V for the full 2048-token batch locally.

v3 notes (vs v2):
- att is pair-major [128, 4, TOK] (head 2p+i at partitions 64i..64i+64, via a
  cross-base DVE copy), so the output projection is 16 full K=128 matmuls
  instead of 64 half-array K=64 ones, and softmax normalize is one [128, TOK]
  multiply per pair.
- Softmax finalization is per-pair-pipelined: denominator scatter + approx
  reciprocal issue at pair end; the reciprocal-broadcast matmul + normalize
  ride the next pair's j-loop as fillers. Tails shrink from ~15us to ~5us.
- reciprocal_approx_fast (~5x faster than InstReciprocal) for denominators.
- K2 pairs 1-2 are produced during att1's finalization tail (PE filler).
- FFN is t-major: all 16 relu'd h1 chunks persist in SBUF (aliasing encT's
  storage), w2 is preloaded into xT's storage; each py[t] accumulates its 16
  matmuls then LNs immediately, overlapping the next t's matmuls.
- Initial loads spread across 4 hardware DMA queues (sync/scalar/vector/gpsimd).
"""
import sys

for _p in ("/opt/trn_rl_repo",):
    if _p not in sys.path:
        sys.path.insert(0, _p)

import numpy as np
import ml_dtypes

import concourse.bass as bass
import concourse.mybir as mybir
import concourse.tile as tile
from concourse import bacc

B, S, D, H, DEPTH, DFF = 2, 2048, 512, 8, 64, 2048
NCORES, G = 8, 4
TOK = 512            # q rows per core
NB = S // 128        # 16 kpos blocks per batch
DT = mybir.dt.float32
DTR = mybir.dt.float32r
BF = mybir.dt.bfloat16
FP = np.float32
BFNP = ml_dtypes.bfloat16


def R(ap):
    """Bitcast an fp32 AP to float32r (same 4-byte layout)."""
    return ap.bitcast(DTR)


WNAMES = [
    "mha1_wq", "mha1_wk", "mha1_wv", "mha1_wo",
    "mha2_wq", "mha2_wk", "mha2_wv", "mha2_wo",
    "ffn_w1", "ffn_w2",
]


def _proj_T(nc, P, w_sb, srcT_sb, out, name, fillers=None):
    """out[:, c, :] (bf16, [128, 4, 512]) = (src @ w).T via 4x4 accumulation."""
    for c in range(4):
        pq = P["psum"].tile([128, 512], DT, tag="pk", bufs=2, name=f"p_{name}")
        for s in range(4):
            nc.tensor.matmul(pq, w_sb[:, s, 128 * c:128 * c + 128],
                             srcT_sb[:, s, :], start=(s == 0), stop=(s == 3))
        nc.vector.tensor_copy(out[:, c, :], pq)
        if fillers:
            fillers.pop(0)()


def _v_chunk(nc, P, srcT_sb, wv_sb, v_sb, ko):
    """V_aug rows [128*ko, 128*ko+128) for all heads into SBUF."""
    pv = P["psum"].tile([128, 512], DT, tag="pk", bufs=2, name="pv")
    for s in range(4):
        nc.tensor.matmul(pv, srcT_sb[:, s, 128 * ko:128 * ko + 128],
                         wv_sb[:, s, :], start=(s == 0), stop=(s == 3))
    nc.vector.tensor_copy(v_sb[:, ko, :, 0:64],
                          pv.rearrange("p (h e) -> p h e", h=H))


def _ln_tile(nc, P, out_ap, psum_ap, resid_ap):
    """out = LN(psum + resid) for one [128, 512] tile (g=1, b=0)."""
    sbuf = P["sbuf"]
    a = sbuf.tile([128, 512], DT, tag="ln_a", bufs=2, name="ln_a")
    nc.vector.tensor_add(a, psum_ap, resid_ap)
    stats = sbuf.tile([128, 6], DT, tag="ln_st", bufs=2, name="ln_st")
    nc.vector.bn_stats(stats, a)
    mv = sbuf.tile([128, 2], DT, tag="ln_mv", bufs=2, name="ln_mv")
    nc.vector.bn_aggr(mv, stats)
    rs = sbuf.tile([128, 1], DT, tag="ln_rs", bufs=2, name="ln_rs")
    nc.scalar.activation(rs, mv[:, 1:2], mybir.ActivationFunctionType.Sqrt,
                         bias=P["eps"])
    nc.vector.reciprocal(rs, rs)
    nc.vector.tensor_scalar(out_ap, a, mv[:, 0:1], rs,
                            op0=mybir.AluOpType.subtract,
                            op1=mybir.AluOpType.mult)


def _transpose_q(nc, P, src_bf, out_bf, fillers=None):
    """src [128, 4, 512] bf16 (token-major) -> out [128, 4, 512] bf16 with
    out[:, s, 128o:128o+128] = src[:, o, 128s:+128].T.  Uses regular matmuls
    against a bf16 identity (1 cyc/row, HAM-warming) instead of transpose mode."""
    for s in range(4):
        pt = P["psum"].tile([128, 512], DT, tag="pk", bufs=2, name="pt")
        for o in range(4):
            nc.tensor.matmul(pt[:, 128 * o:128 * o + 128],
                             src_bf[:, o, 128 * s:128 * s + 128],
                             P["identity"], start=True, stop=True,
                             skip_group_check=True)
        nc.vector.tensor_copy(out_bf[:, s, :], pt)
        if fillers:
            fillers.pop(0)()


def _kprod_chunks(nc, P, wk_sb, srcT_sb, pair, key):
    """4 callables, each producing one 512-column chunk of K^T for this head
    pair into the shared kt ring tile (allocated at first chunk)."""
    def chunk(c):
        def fn():
            if key not in P["kt_store"]:
                P["kt_store"][key] = P["sbuf"].tile(
                    [128, S], BF, tag="kt", bufs=3, name="kTp")
            kTp = P["kt_store"][key]
            pkk = P["psum"].tile([128, 512], DT, tag="pk", bufs=2, name="pkk")
            for s in range(4):
                nc.tensor.matmul(pkk, wk_sb[:, s, 128 * pair:128 * pair + 128],
                                 srcT_sb[:, s, 512 * c:512 * c + 512],
                                 start=(s == 0), stop=(s == 3))
            nc.vector.tensor_copy(kTp[:, 512 * c:512 * c + 512], pkk)
        return fn
    return [chunk(c) for c in range(4)]


def _attention(nc, P, qT_sb, srcT_sb, wk_sb, v_sb, causal, wo_sb,
               resid_sb, out_fp32, out_bf16, kpfx, pair_fillers,
               tail_fillers=()):
    """Transposed-layout attention + output projection + residual + LN.
    att is pair-major: head 2p+i lives at partitions 64i..64i+64 of att[:, p].
    Per-pair finalization (denominator recip + broadcast + normalize) pipelines
    into the next pair's j-loop; O-proj is K=128 pair-stacked matmuls."""
    sbuf = P["sbuf"]
    att = sbuf.tile([128, H // 2, TOK], BF, tag="att", bufs=1, name="att")
    sums8 = sbuf.tile([8, TOK], DT, tag="sums8", bufs=1, name="sums8")
    rec8 = sbuf.tile([8, TOK], DT, tag="rec8", bufs=1, name="rec8")
    rec8r = sbuf.tile([8, TOK], DTR, tag="rec8r", bufs=1, name="rec8r")
    nc.vector.memset(sums8, 1.0)

    def finalize_a(pair, av):
        """Issue at pair end: denominator scatter + (whole-tile) reciprocal,
        and the pair-major PSUM->SBUF evacuation of attention values."""
        srow = sbuf.tile([65, 2, TOK], DT, tag="srow", bufs=2, name="srow")
        nc.vector.tensor_copy(srow[64:65, :, :], av[64:65, :, :])
        nc.gpsimd.dma_start(sums8[2 * pair:2 * pair + 2, :],
                            srow[64:65, :, :])
        nc.vector.tensor_copy(att[0:64, pair, :], av[0:64, 0, :])
        nc.vector.tensor_copy(att[64:128, pair, :], av[0:64, 1, :])
        # recip of the full tile: rows of later pairs are 1.0 (memset) and get
        # recomputed when their sums land; this pair's rows are now valid
        nc.vector.reciprocal_approx_fast(rec8, sums8)
        with nc.allow_low_precision(reason="fp32r rounding for PE broadcast"):
            nc.vector.tensor_copy(rec8r, rec8)

    def finalize_b(pair):
        """Broadcast 1/sum across 128 partitions (one M=128 matmul via the
        pair-selector) and normalize att[:, pair] in place. Runs as a filler
        inside the NEXT pair's j-loop so the PE never stalls on the recip."""
        def fn():
            bcp = P["psum"].tile([128, TOK], DT, tag="pk", bufs=2, name="bcp")
            nc.tensor.matmul(bcp, P["ind8"][:, pair, :], rec8r,
                             start=True, stop=True)
            nc.vector.tensor_mul(att[:, pair, :], att[:, pair, :], bcp)
        return fn

    for pair in range(H // 2):
        if (kpfx, pair) not in P["kt_store"]:
            for fn in _kprod_chunks(nc, P, wk_sb, srcT_sb, pair,
                                    (kpfx, pair)):
                fn()
        kTp = P["kt_store"][(kpfx, pair)]
        fillers = list(pair_fillers[pair])
        if pair > 0:
            fillers.insert(0, finalize_b(pair - 1))
        if pair + 1 < H // 2:
            fillers += _kprod_chunks(nc, P, wk_sb, srcT_sb, pair + 1,
                                     (kpfx, pair + 1))

        av = P["psum"].tile([65, 2, TOK], DT, tag="av", bufs=1, name="av")
        for j in range(NB):
            c0 = 32 * j if causal else 0
            st = P["psum"].tile([128, 2, TOK], DT, tag="st", bufs=2, name="st")
            for i in range(2):
                hp = 64 * i
                nc.tensor.matmul(st[:, i, c0:],
                                 kTp[hp:hp + 64, 128 * j:128 * j + 128],
                                 qT_sb[hp:hp + 64, pair, c0:],
                                 start=True, stop=True)
            pp = sbuf.tile([128, 2, TOK], BF, tag="pp", bufs=3, name="pp")
            # exp(S / sqrt(DEPTH)): scale fused into the activation
            nc.scalar.activation(pp[:, :, c0:], st[:, :, c0:],
                                 mybir.ActivationFunctionType.Exp,
                                 scale=float(1.0 / np.sqrt(DEPTH)))
            if causal:
                m = P["mask"][:, None, :].to_broadcast([128, 2, 32])
                nc.vector.tensor_mul(pp[:, :, c0:c0 + 32],
                                     pp[:, :, c0:c0 + 32], m)
            for i in range(2):
                nc.tensor.matmul(av[:, i, c0:], v_sb[:, j, 2 * pair + i, :],
                                 pp[:, i, c0:],
                                 start=(j == 0), stop=(j == NB - 1),
                                 skip_group_check=True)
            if j % 2 == 1 and fillers:
                fillers.pop(0)()
        while fillers:
            fillers.pop(0)()
        finalize_a(pair, av)

    tail = list(tail_fillers)
    mid = (len(tail) + 1) // 2
    for fn in tail[:mid]:
        fn()
    finalize_b(H // 2 - 1)()
    for fn in tail[mid:]:
        fn()

    # O-projection: 4 pair-stacked K=128 matmuls per token tile, then LN
    for t in range(4):
        po = P["psum"].tile([128, 512], DT, tag="pk", bufs=2, name="po")
        for p in range(H // 2):
            nc.tensor.matmul(po, att[:, p, 128 * t:128 * t + 128],
                             wo_sb[:, p, :], start=(p == 0), stop=(p == 3))
        _ln_tile(nc, P, out_fp32[:, t, :], po, resid_sb[:, t, :])
        nc.scalar.copy(out_bf16[:, t, :], out_fp32[:, t, :])


def build_program():
    nc = bacc.Bacc("TRN2", num_devices=NCORES)
    io = {}
    io["xT"] = nc.dram_tensor("xT", [D, S], BF, kind="ExternalInput")
    io["encT"] = nc.dram_tensor("encT", [D, S], BF, kind="ExternalInput")
    io["xqT"] = nc.dram_tensor("xqT", [D, TOK], BF, kind="ExternalInput")
    io["x_q"] = nc.dram_tensor("x_q", [TOK, D], DT, kind="ExternalInput")
    io["diagmask"] = nc.dram_tensor("diagmask", [128, 32], BF,
                                    kind="ExternalInput")
    io["ident128"] = nc.dram_tensor("ident128", [128, 128], BF,
                                    kind="ExternalInput")
    io["ind8"] = nc.dram_tensor("ind8", [8, D], DT, kind="ExternalInput")
    for w in WNAMES:
        shp = [D, DFF] if w == "ffn_w1" else ([DFF, D] if w == "ffn_w2" else [D, D])
        io[w] = nc.dram_tensor(w, shp, BF, kind="ExternalInput")
    io["y"] = nc.dram_tensor("y", [TOK, D], DT, kind="ExternalOutput")

    with tile.TileContext(nc) as tc:
        import contextlib
        with contextlib.ExitStack() as ctx:
            P = {}
            P["sbuf"] = ctx.enter_context(tc.tile_pool(name="sbuf", bufs=1))
            P["wsb"] = ctx.enter_context(tc.tile_pool(name="wsb", bufs=1))
            P["consts"] = ctx.enter_context(tc.tile_pool(name="consts", bufs=1))
            P["psum"] = ctx.enter_context(
                tc.tile_pool(name="psum", bufs=2, space="PSUM"))
            sbuf = P["sbuf"]
            wsb = P["wsb"]

            eps_tile = P["consts"].tile([128, 1], DT, tag="eps", name="eps")
            nc.vector.memset(eps_tile, 1e-6)
            P["eps"] = eps_tile

            # ---- initial loads spread over 4 hardware DMA queues ----
            def w_load(eng, name, tag):
                t = wsb.tile([128, 4, 512], BF, tag=tag, bufs=1, name=tag)
                eng.dma_start(t, io[name].rearrange("(s p) n -> p s n", p=128))
                return t

            def wo_load(eng, name, tag):
                # pair-major: partition q = 64i + d for head 2p+i; free = pair
                t = wsb.tile([128, 4, 512], BF, tag=tag, bufs=1, name=tag)
                eng.dma_start(t, io[name].rearrange("(pr q) n -> q pr n", q=128))
                return t

            # sync queue: Q1's inputs first, then xT halves, wo1, identity
            xqT_sb = sbuf.tile([128, 4, TOK], BF, tag="xqT", bufs=1,
                               name="xqT_sb")
            xqT_ap = io["xqT"].rearrange("(s p) t -> p s t", p=128)
            nc.sync.dma_start(xqT_sb[:, :, 0:256], xqT_ap[:, :, 0:256])
            wq1_sb = w_load(nc.scalar, "mha1_wq", "wq1")
            nc.scalar.dma_start(xqT_sb[:, :, 256:512], xqT_ap[:, :, 256:512])
            xT_sb = sbuf.tile([128, 4, S], BF, tag="xT", bufs=1, name="xT_sb")
            xT_ap = io["xT"].rearrange("(s p) t -> p s t", p=128)
            for q in range(2):
                nc.sync.dma_start(xT_sb[:, :, 512 * q:512 * q + 512],
                                  xT_ap[:, :, 512 * q:512 * q + 512])
            # scalar queue: V1/K1 weights, xT upper half, residual
            wv1_sb = w_load(nc.scalar, "mha1_wv", "wv1")
            wk1_sb = w_load(nc.scalar, "mha1_wk", "wk1")
            for q in range(2, 4):
                nc.scalar.dma_start(xT_sb[:, :, 512 * q:512 * q + 512],
                                    xT_ap[:, :, 512 * q:512 * q + 512])
            # gpsimd queue: small consts, encT, mha2 weights (and later the
            # in-loop denominator scatters — nothing here may block on a
            # late-released tile, or those scatters would deadlock att1)
            mask_sb = P["consts"].tile([128, 32], BF, name="mask_sb")
            nc.gpsimd.dma_start(mask_sb, io["diagmask"][:, :])
            P["mask"] = mask_sb
            ind8 = P["consts"].tile([8, 4, 128], DTR, tag="ind8", name="ind8")
            nc.gpsimd.dma_start(ind8, R(io["ind8"])
                                .rearrange("k (p c) -> k p c", p=4))
            P["ind8"] = ind8
            encT_sb = sbuf.tile([128, 4, S], BF, tag="encT", bufs=1,
                                name="encT_sb")
            encT_ap = io["encT"].rearrange("(s p) t -> p s t", p=128)
            for q in range(4):
                nc.gpsimd.dma_start(encT_sb[:, :, 512 * q:512 * q + 512],
                                    encT_ap[:, :, 512 * q:512 * q + 512])
            wv2_sb = w_load(nc.gpsimd, "mha2_wv", "wv2")
            wk2_sb = w_load(nc.gpsimd, "mha2_wk", "wk2")
            wo2_sb = wo_load(nc.gpsimd, "mha2_wo", "wo2")
            wq2_sb = w_load(nc.gpsimd, "mha2_wq", "wq2")
            # stragglers
            wo1_sb = wo_load(nc.sync, "mha1_wo", "wo1")
            identity = P["consts"].tile([128, 128], BF, name="identity")
            nc.sync.dma_start(identity, io["ident128"][:, :])
            P["identity"] = identity
            xq_res = sbuf.tile([128, 4, D], DT, tag="xq_res", bufs=1,
                               name="xq_res")
            nc.scalar.dma_start(xq_res,
                                io["x_q"].rearrange("(o p) d -> p o d", p=128))
            # w2 preload into xT's storage (free after att1). Its DMA waits on
            # xT's last reader, so it sits LAST on the scalar queue before the
            # FFN w1c stream (which isn't needed until long after).
            w2_all = sbuf.tile([128, 4, S], BF, tag="xT", bufs=1,
                               name="w2_all")
            w2_ap = io["ffn_w2"].rearrange("(b a p) n -> p a b n", a=4, p=128)
            for b in range(4):
                nc.sync.dma_start(w2_all[:, :, 512 * b:512 * b + 512],
                                  w2_ap[:, :, b, :])
            P["kt_store"] = {}

            # ---------------- Q1 projection + V1 ----------------
            qT1 = sbuf.tile([128, 4, TOK], BF, tag="qT1", bufs=1, name="qT1")
            _proj_T(nc, P, wq1_sb, xqT_sb, qT1, "q1")
            v1_sb = sbuf.tile([128, NB, H, 65], BF, tag="v1", bufs=1,
                              name="v1_sb")
            nc.vector.memset(v1_sb[:, :, :, 64], 1.0)
            for ko in range(NB):
                _v_chunk(nc, P, xT_sb, wv1_sb, v1_sb, ko)

            # ---------------- mha1 (causal self-attention) ----------------
            # v2 production is issued inside att1's pair loop: independent PE
            # work that runs while ScalarE drains att1's softmax exps.
            v2_sb = sbuf.tile([128, NB, H, 65], BF, tag="v2", bufs=1,
                              name="v2_sb")
            nc.vector.memset(v2_sb[:, :, :, 64], 1.0)

            def v2_chunks(lo, hi):
                return [(lambda ko: (lambda: _v_chunk(
                    nc, P, encT_sb, wv2_sb, v2_sb, ko)))(ko)
                    for ko in range(lo, hi)]

            out1 = sbuf.tile([128, 4, D], DT, tag="out1", bufs=1, name="out1")
            out1b = sbuf.tile([128, 4, D], BF, tag="out1b", bufs=1,
                              name="out1b")
            att1_fillers = [v2_chunks(2 * q, 2 * q + 2) for q in range(4)]
            # att1 pair 3 additionally pre-builds att2 pair 0's K^T
            att1_fillers[3] = att1_fillers[3] + _kprod_chunks(
                nc, P, wk2_sb, encT_sb, 0, ("k2", 0))
            # K2 pairs 1-2 fill att1's finalization tail
            att1_tail = (_kprod_chunks(nc, P, wk2_sb, encT_sb, 1, ("k2", 1))
                         + _kprod_chunks(nc, P, wk2_sb, encT_sb, 2, ("k2", 2)))
            _attention(nc, P, qT1, xT_sb, wk1_sb, v1_sb, True, wo1_sb,
                       xq_res, out1, out1b, "k1", att1_fillers, att1_tail)

            # ---------------- out1^T, Q2 ----------------
            out1T = sbuf.tile([128, 4, TOK], BF, tag="out1T", bufs=1,
                              name="out1T")
            v2_rest = v2_chunks(8, 16)
            _transpose_q(nc, P, out1b, out1T, fillers=v2_rest)
            qT2 = sbuf.tile([128, 4, TOK], BF, tag="qT2", bufs=1, name="qT2")
            _proj_T(nc, P, wq2_sb, out1T, qT2, "q2", fillers=v2_rest)
            while v2_rest:
                v2_rest.pop(0)()

            # ---------------- mha2 (cross-attention, no mask) --------------
            out2 = sbuf.tile([128, 4, D], DT, tag="out2", bufs=1, name="out2")
            out2b = sbuf.tile([128, 4, D], BF, tag="out2b", bufs=1,
                              name="out2b")
            _attention(nc, P, qT2, encT_sb, wk2_sb, v2_sb, False, wo2_sb,
                       out1, out2, out2b, "k2", [[], [], [], []])

            # ---------------- FFN ----------------
            out2T = sbuf.tile([128, 4, TOK], BF, tag="out2T", bufs=1,
                              name="out2T")
            _transpose_q(nc, P, out2b, out2T)
            # h1 = relu(out2 @ w1): all 16 chunks persist (aliases encT)
            h1_all = sbuf.tile([128, 4, S], BF, tag="encT", bufs=1,
                               name="h1_all")
            for c in range(DFF // 128):
                w1c = wsb.tile([128, 4, 128], BF, tag="w1c", bufs=4,
                               name="w1c")
                eng = nc.sync if c % 2 == 0 else nc.scalar
                eng.dma_start(
                    w1c, io["ffn_w1"][:, 128 * c:128 * c + 128]
                    .rearrange("(s p) n -> p s n", p=128))
                ph = P["psum"].tile([128, 512], DT, tag="pk", bufs=2,
                                    name="ph")
                for s in range(4):
                    nc.tensor.matmul(ph, w1c[:, s, :], out2T[:, s, :],
                                     start=(s == 0), stop=(s == 3))
                nc.scalar.activation(
                    h1_all[:, c % 4, 512 * (c // 4):512 * (c // 4) + 512], ph,
                    mybir.ActivationFunctionType.Relu)
            # y = h1 @ w2, t-major so each LN overlaps the next t's matmuls
            py0 = P["psum"].tile([128, 2, 512], DT, tag="st", bufs=2,
                                 name="py0")
            py1 = P["psum"].tile([128, 2, 512], DT, tag="st", bufs=2,
                                 name="py1")
            py = [py0, py1]
            for t in range(4):
                pyt = py[t // 2][:, t % 2, :]
                for c in range(DFF // 128):
                    a, b = c % 4, c // 4
                    nc.tensor.matmul(
                        pyt,
                        h1_all[:, a, 512 * b + 128 * t:512 * b + 128 * t + 128],
                        w2_all[:, a, 512 * b:512 * b + 512],
                        start=(c == 0), stop=(c == DFF // 128 - 1),
                        skip_group_check=True)
                ya = sbuf.tile([128, 512], DT, tag="ya", bufs=2, name="ya")
                _ln_tile(nc, P, ya, pyt, out2[:, t, :])
                nc.sync.dma_start(
                    io["y"].rearrange("(o p) d -> p o d", p=128)[:, t, :], ya)
    nc.compile()
    return nc


_CACHED = None


def _get_program():
    global _CACHED
    if _CACHED is None:
        _CACHED = build_program()
    return _CACHED


def make_in_maps(inputs):
    x = np.ascontiguousarray(np.asarray(inputs["x"], FP))
    enc = np.ascontiguousarray(np.asarray(inputs["enc_output"], FP))
    lam = np.asarray(inputs["look_ahead_mask"], FP)
    pad = np.asarray(inputs["padding_mask"], FP)
    assert np.array_equal(lam[0, 0], np.triu(np.ones((S, S), FP), k=1)), \
        "kernel specialized for causal look_ahead_mask"
    assert not pad.any(), "kernel specialized for zero padding_mask"
    for p in ("mha1", "mha2"):
        for nm in ("q", "k", "v", "o"):
            assert not np.asarray(inputs[f"{p}_b{nm}"], FP).any()
    assert not np.asarray(inputs["ffn_b1"], FP).any()
    assert not np.asarray(inputs["ffn_b2"], FP).any()
    for i in (1, 2, 3):
        assert np.all(np.asarray(inputs[f"ln{i}_g"], FP) == 1.0)
        assert not np.asarray(inputs[f"ln{i}_b"], FP).any()

    weights = {w: np.ascontiguousarray(np.asarray(inputs[w], FP).astype(BFNP))
               for w in WNAMES}
    xT = [np.ascontiguousarray(x[b].T.astype(BFNP)) for b in range(B)]
    encT = [np.ascontiguousarray(enc[b].T.astype(BFNP)) for b in range(B)]
    in_maps = []
    for c in range(NCORES):
        b, r = c // G, c % G
        kp = np.arange(128)[:, None]
        u = np.arange(32)[None, :]
        M = (kp <= 4 * u + r).astype(BFNP)
        m = {
            "ident128": np.eye(128, dtype=BFNP),
            "ind8": np.kron(np.eye(8, dtype=FP), np.ones((1, 64), FP)),
            "xT": xT[b],
            "encT": encT[b],
            "xqT": np.ascontiguousarray(xT[b][:, r::4]),
            "x_q": np.ascontiguousarray(x[b, r::4, :]),
            "diagmask": M,
        }
        m.update(weights)
        in_maps.append(m)
    return in_maps


def kernel(**inputs):
    from concourse.bass_utils import run_bass_kernel_spmd
    nc = _get_program()
    in_maps = make_in_maps(inputs)
    res = run_bass_kernel_spmd(nc, in_maps, list(range(NCORES)))
    out = np.empty((B, S, D), FP)
    for c in range(NCORES):
        b, r = c // G, c % G
        out[b, r::4, :] = res.results[c]["y"]
    return out


# revision 14
# speedup vs baseline: 1.0085x; 1.0085x over previous
"""Trainium2 Bass kernel for a transformer decoder layer (self-attn + cross-attn + FFN).

Sharding (8 cores): sequence-parallel, no collectives. Cores 0-3 handle batch 0,
cores 4-7 batch 1. The core with group-rank r owns query rows r::4 of its batch
(interleaved, so the causal workload is identical on every core). Every core
computes K/V for the full 2048-token batch locally.

v3 notes (vs v2):
- att is pair-major [128, 4, TOK] (head 2p+i at partitions 64i..64i+64, via a
  cross-base DVE copy), so the output projection is 16 full K=128 matmuls
  instead of 64 half-array K=64 ones, and softmax normalize is one [128, TOK]
  multiply per pair.
- Softmax finalization is per-pair-pipelined: denominator scatter + approx
  reciprocal issue at pair end; the reciprocal-broadcast matmul + normalize
  ride the next pair's j-loop as fillers. Tails shrink from ~15us to ~5us.
- reciprocal_approx_fast (~5x faster than InstReciprocal) for denominators.
- K2 pairs 1-2 are produced during att1's finalization tail (PE filler).
- FFN is t-major: all 16 relu'd h1 chunks persist in SBUF (aliasing encT's
  storage), w2 is preloaded into xT's storage; each py[t] accumulates its 16
  matmuls then LNs immediately, overlapping the next t's matmuls.
- Initial loads spread across 4 hardware DMA queues (sync/scalar/vector/gpsimd).
"""
import sys

for _p in ("/opt/trn_rl_repo",):
    if _p not in sys.path:
        sys.path.insert(0, _p)

import numpy as np
import ml_dtypes

import concourse.bass as bass
import concourse.mybir as mybir
import concourse.tile as tile
from concourse import bacc

B, S, D, H, DEPTH, DFF = 2, 2048, 512, 8, 64, 2048
NCORES, G = 8, 4
TOK = 512            # q rows per core
NB = S // 128        # 16 kpos blocks per batch
DT = mybir.dt.float32
DTR = mybir.dt.float32r
BF = mybir.dt.bfloat16
FP = np.float32
BFNP = ml_dtypes.bfloat16


def R(ap):
    """Bitcast an fp32 AP to float32r (same 4-byte layout)."""
    return ap.bitcast(DTR)


WNAMES = [
    "mha1_wq", "mha1_wk", "mha1_wv", "mha1_wo",
    "mha2_wq", "mha2_wk", "mha2_wv", "mha2_wo",
    "ffn_w1", "ffn_w2",
]


def _proj_T(nc, P, w_sb, srcT_sb, out, name):
    """out[:, c, :] (bf16, [128, 4, 512]) = (src @ w).T via 4x4 accumulation."""
    for c in range(4):
        pq = P["psum"].tile([128, 512], DT, tag="pk", bufs=2, name=f"p_{name}")
        for s in range(4):
            nc.tensor.matmul(pq, w_sb[:, s, 128 * c:128 * c + 128],
                             srcT_sb[:, s, :], start=(s == 0), stop=(s == 3))
        nc.vector.tensor_copy(out[:, c, :], pq)


def _v_chunk(nc, P, srcT_sb, wv_sb, v_sb, ko):
    """V_aug rows [128*ko, 128*ko+128) for all heads into SBUF."""
    pv = P["psum"].tile([128, 512], DT, tag="pk", bufs=2, name="pv")
    for s in range(4):
        nc.tensor.matmul(pv, srcT_sb[:, s, 128 * ko:128 * ko + 128],
                         wv_sb[:, s, :], start=(s == 0), stop=(s == 3))
    nc.vector.tensor_copy(v_sb[:, ko, :, 0:64],
                          pv.rearrange("p (h e) -> p h e", h=H))


def _ln_tile(nc, P, out_ap, psum_ap, resid_ap):
    """out = LN(psum + resid) for one [128, 512] tile (g=1, b=0)."""
    sbuf = P["sbuf"]
    a = sbuf.tile([128, 512], DT, tag="ln_a", bufs=2, name="ln_a")
    nc.vector.tensor_add(a, psum_ap, resid_ap)
    stats = sbuf.tile([128, 6], DT, tag="ln_st", bufs=2, name="ln_st")
    nc.vector.bn_stats(stats, a)
    mv = sbuf.tile([128, 2], DT, tag="ln_mv", bufs=2, name="ln_mv")
    nc.vector.bn_aggr(mv, stats)
    rs = sbuf.tile([128, 1], DT, tag="ln_rs", bufs=2, name="ln_rs")
    nc.scalar.activation(rs, mv[:, 1:2], mybir.ActivationFunctionType.Sqrt,
                         bias=P["eps"])
    nc.vector.reciprocal(rs, rs)
    nc.vector.tensor_scalar(out_ap, a, mv[:, 0:1], rs,
                            op0=mybir.AluOpType.subtract,
                            op1=mybir.AluOpType.mult)


def _transpose_q(nc, P, src_bf, out_bf, fillers=None):
    """src [128, 4, 512] bf16 (token-major) -> out [128, 4, 512] bf16 with
    out[:, s, 128o:128o+128] = src[:, o, 128s:+128].T.  Uses regular matmuls
    against a bf16 identity (1 cyc/row, HAM-warming) instead of transpose mode."""
    for s in range(4):
        pt = P["psum"].tile([128, 512], DT, tag="pk", bufs=2, name="pt")
        for o in range(4):
            nc.tensor.matmul(pt[:, 128 * o:128 * o + 128],
                             src_bf[:, o, 128 * s:128 * s + 128],
                             P["identity"], start=True, stop=True,
                             skip_group_check=True)
        nc.vector.tensor_copy(out_bf[:, s, :], pt)
        if fillers:
            fillers.pop(0)()


def _kprod_chunks(nc, P, wk_sb, srcT_sb, pair, key):
    """4 callables, each producing one 512-column chunk of K^T for this head
    pair into the shared kt ring tile (allocated at first chunk)."""
    def chunk(c):
        def fn():
            if key not in P["kt_store"]:
                P["kt_store"][key] = P["sbuf"].tile(
                    [128, S], BF, tag="kt", bufs=4, name="kTp")
            kTp = P["kt_store"][key]
            pkk = P["psum"].tile([128, 512], DT, tag="pk", bufs=2, name="pkk")
            for s in range(4):
                nc.tensor.matmul(pkk, wk_sb[:, s, 128 * pair:128 * pair + 128],
                                 srcT_sb[:, s, 512 * c:512 * c + 512],
                                 start=(s == 0), stop=(s == 3))
            nc.vector.tensor_copy(kTp[:, 512 * c:512 * c + 512], pkk)
        return fn
    return [chunk(c) for c in range(4)]


def _attention(nc, P, qT_sb, srcT_sb, wk_sb, v_sb, causal, wo_sb,
               resid_sb, out_fp32, out_bf16, kpfx, pair_fillers,
               tail_fillers=()):
    """Transposed-layout attention + output projection + residual + LN.
    att is pair-major: head 2p+i lives at partitions 64i..64i+64 of att[:, p].
    Per-pair finalization (denominator recip + broadcast + normalize) pipelines
    into the next pair's j-loop; O-proj is K=128 pair-stacked matmuls."""
    sbuf = P["sbuf"]
    att = sbuf.tile([128, H // 2, TOK], BF, tag="att", bufs=1, name="att")
    sums8 = sbuf.tile([8, TOK], DT, tag="sums8", bufs=1, name="sums8")
    rec8 = sbuf.tile([8, TOK], DT, tag="rec8", bufs=1, name="rec8")
    rec8r = sbuf.tile([8, TOK], DTR, tag="rec8r", bufs=1, name="rec8r")
    nc.vector.memset(sums8, 1.0)

    def finalize_a(pair, av):
        """Issue at pair end: denominator scatter + (whole-tile) reciprocal,
        and the pair-major PSUM->SBUF evacuation of attention values."""
        srow = sbuf.tile([65, 2, TOK], DT, tag="srow", bufs=2, name="srow")
        if pair == H // 2 - 1:
            # exposed tail: halve the chain head; ScalarE is idle here
            nc.vector.tensor_copy(srow[64:65, 0:1, :], av[64:65, 0:1, :])
            nc.scalar.copy(srow[64:65, 1:2, :], av[64:65, 1:2, :])
        else:
            nc.vector.tensor_copy(srow[64:65, :, :], av[64:65, :, :])
        nc.gpsimd.dma_start(sums8[2 * pair:2 * pair + 2, :],
                            srow[64:65, :, :])
        nc.vector.tensor_copy(att[0:64, pair, :], av[0:64, 0, :])
        nc.vector.tensor_copy(att[64:128, pair, :], av[0:64, 1, :])
        # recip of the full tile: rows of later pairs are 1.0 (memset) and get
        # recomputed when their sums land; this pair's rows are now valid
        nc.vector.reciprocal_approx_fast(rec8, sums8)
        with nc.allow_low_precision(reason="fp32r rounding for PE broadcast"):
            nc.vector.tensor_copy(rec8r, rec8)

    def finalize_b(pair):
        """Broadcast 1/sum across 128 partitions (one M=128 matmul via the
        pair-selector) and normalize att[:, pair] in place. Runs as a filler
        inside the NEXT pair's j-loop so the PE never stalls on the recip."""
        def fn():
            bcp = P["psum"].tile([128, TOK], DT, tag="pk", bufs=2, name="bcp")
            nc.tensor.matmul(bcp, P["ind8"][:, pair, :], rec8r,
                             start=True, stop=True)
            nc.vector.tensor_mul(att[:, pair, :], att[:, pair, :], bcp)
        return fn

    for pair in range(H // 2):
        if (kpfx, pair) not in P["kt_store"]:
            for fn in _kprod_chunks(nc, P, wk_sb, srcT_sb, pair,
                                    (kpfx, pair)):
                fn()
        kTp = P["kt_store"][(kpfx, pair)]
        fillers = list(pair_fillers[pair])
        if pair > 0:
            fillers.insert(0, finalize_b(pair - 1))
        if pair + 1 < H // 2 and (kpfx, pair + 1) not in P["kt_store"]:
            fillers += _kprod_chunks(nc, P, wk_sb, srcT_sb, pair + 1,
                                     (kpfx, pair + 1))

        av = P["psum"].tile([65, 2, TOK], DT, tag="av", bufs=1, name="av")
        for j in range(NB):
            c0 = 32 * j if causal else 0
            st = P["psum"].tile([128, 2, TOK], DT, tag="st", bufs=2, name="st")
            for i in range(2):
                hp = 64 * i
                nc.tensor.matmul(st[:, i, c0:],
                                 kTp[hp:hp + 64, 128 * j:128 * j + 128],
                                 qT_sb[hp:hp + 64, pair, c0:],
                                 start=True, stop=True)
            pp = sbuf.tile([128, 2, TOK], BF, tag="pp", bufs=4, name="pp")
            # exp(S / sqrt(DEPTH)): scale fused into the activation
            nc.scalar.activation(pp[:, :, c0:], st[:, :, c0:],
                                 mybir.ActivationFunctionType.Exp,
                                 scale=float(1.0 / np.sqrt(DEPTH)))
            if causal:
                m = P["mask"][:, None, :].to_broadcast([128, 2, 32])
                nc.vector.tensor_mul(pp[:, :, c0:c0 + 32],
                                     pp[:, :, c0:c0 + 32], m)
            for i in range(2):
                nc.tensor.matmul(av[:, i, c0:], v_sb[:, j, 2 * pair + i, :],
                                 pp[:, i, c0:],
                                 start=(j == 0), stop=(j == NB - 1),
                                 skip_group_check=True)
            if j % 2 == 1 and fillers:
                fillers.pop(0)()
        while fillers:
            fillers.pop(0)()
        finalize_a(pair, av)

    tail = list(tail_fillers)
    mid = (len(tail) + 1) // 2
    for fn in tail[:mid]:
        fn()
    finalize_b(H // 2 - 1)()
    for fn in tail[mid:]:
        fn()

    # O-projection: 4 pair-stacked K=128 matmuls per token tile, then LN
    for t in range(4):
        po = P["psum"].tile([128, 512], DT, tag="pk", bufs=2, name="po")
        for p in range(H // 2):
            nc.tensor.matmul(po, att[:, p, 128 * t:128 * t + 128],
                             wo_sb[:, p, :], start=(p == 0), stop=(p == 3))
        _ln_tile(nc, P, out_fp32[:, t, :], po, resid_sb[:, t, :])
        nc.scalar.copy(out_bf16[:, t, :], out_fp32[:, t, :])


def build_program():
    nc = bacc.Bacc("TRN2", num_devices=NCORES)
    io = {}
    io["xT"] = nc.dram_tensor("xT", [D, S], BF, kind="ExternalInput")
    io["encT"] = nc.dram_tensor("encT", [D, S], BF, kind="ExternalInput")
    io["xqT"] = nc.dram_tensor("xqT", [D, TOK], BF, kind="ExternalInput")
    io["x_q"] = nc.dram_tensor("x_q", [TOK, D], DT, kind="ExternalInput")
    io["diagmask"] = nc.dram_tensor("diagmask", [128, 32], BF,
                                    kind="ExternalInput")
    io["ident128"] = nc.dram_tensor("ident128", [128, 128], BF,
                                    kind="ExternalInput")
    io["ind8"] = nc.dram_tensor("ind8", [8, D], DT, kind="ExternalInput")
    for w in WNAMES:
        shp = [D, DFF] if w == "ffn_w1" else ([DFF, D] if w == "ffn_w2" else [D, D])
        io[w] = nc.dram_tensor(w, shp, BF, kind="ExternalInput")
    io["y"] = nc.dram_tensor("y", [TOK, D], DT, kind="ExternalOutput")

    with tile.TileContext(nc) as tc:
        import contextlib
        with contextlib.ExitStack() as ctx:
            P = {}
            P["sbuf"] = ctx.enter_context(tc.tile_pool(name="sbuf", bufs=1))
            P["wsb"] = ctx.enter_context(tc.tile_pool(name="wsb", bufs=1))
            P["consts"] = ctx.enter_context(tc.tile_pool(name="consts", bufs=1))
            P["psum"] = ctx.enter_context(
                tc.tile_pool(name="psum", bufs=2, space="PSUM"))
            sbuf = P["sbuf"]
            wsb = P["wsb"]

            eps_tile = P["consts"].tile([128, 1], DT, tag="eps", name="eps")
            nc.vector.memset(eps_tile, 1e-6)
            P["eps"] = eps_tile

            # ---- initial loads spread over 4 hardware DMA queues ----
            def w_load(eng, name, tag):
                t = wsb.tile([128, 4, 512], BF, tag=tag, bufs=1, name=tag)
                eng.dma_start(t, io[name].rearrange("(s p) n -> p s n", p=128))
                return t

            def wo_load(eng, name, tag):
                # pair-major: partition q = 64i + d for head 2p+i; free = pair
                t = wsb.tile([128, 4, 512], BF, tag=tag, bufs=1, name=tag)
                eng.dma_start(t, io[name].rearrange("(pr q) n -> q pr n", q=128))
                return t

            # sync queue: Q1's inputs first, then xT halves, wo1, identity
            xqT_sb = sbuf.tile([128, 4, TOK], BF, tag="xqT", bufs=1,
                               name="xqT_sb")
            xqT_ap = io["xqT"].rearrange("(s p) t -> p s t", p=128)
            nc.sync.dma_start(xqT_sb[:, :, 0:256], xqT_ap[:, :, 0:256])
            nc.scalar.dma_start(xqT_sb[:, :, 256:512], xqT_ap[:, :, 256:512])
            wq1_sb = w_load(nc.sync, "mha1_wq", "wq1")
            xT_sb = sbuf.tile([128, 4, S], BF, tag="xT", bufs=1, name="xT_sb")
            xT_ap = io["xT"].rearrange("(s p) t -> p s t", p=128)
            for q in range(2):
                nc.sync.dma_start(xT_sb[:, :, 512 * q:512 * q + 512],
                                  xT_ap[:, :, 512 * q:512 * q + 512])
            # scalar queue: V1/K1 weights, xT upper half, residual
            wv1_sb = w_load(nc.scalar, "mha1_wv", "wv1")
            wk1_sb = w_load(nc.scalar, "mha1_wk", "wk1")
            for q in range(2, 4):
                nc.scalar.dma_start(xT_sb[:, :, 512 * q:512 * q + 512],
                                    xT_ap[:, :, 512 * q:512 * q + 512])
            # gpsimd queue: small consts, encT, mha2 weights (and later the
            # in-loop denominator scatters — nothing here may block on a
            # late-released tile, or those scatters would deadlock att1)
            mask_sb = P["consts"].tile([128, 32], BF, name="mask_sb")
            nc.gpsimd.dma_start(mask_sb, io["diagmask"][:, :])
            P["mask"] = mask_sb
            ind8 = P["consts"].tile([8, 4, 128], DTR, tag="ind8", name="ind8")
            nc.gpsimd.dma_start(ind8, R(io["ind8"])
                                .rearrange("k (p c) -> k p c", p=4))
            P["ind8"] = ind8
            encT_sb = sbuf.tile([128, 4, S], BF, tag="encT", bufs=1,
                                name="encT_sb")
            encT_ap = io["encT"].rearrange("(s p) t -> p s t", p=128)
            for q in range(4):
                nc.gpsimd.dma_start(encT_sb[:, :, 512 * q:512 * q + 512],
                                    encT_ap[:, :, 512 * q:512 * q + 512])
            wv2_sb = w_load(nc.gpsimd, "mha2_wv", "wv2")
            wk2_sb = w_load(nc.gpsimd, "mha2_wk", "wk2")
            wo2_sb = wo_load(nc.gpsimd, "mha2_wo", "wo2")
            wq2_sb = w_load(nc.gpsimd, "mha2_wq", "wq2")
            # stragglers
            wo1_sb = wo_load(nc.sync, "mha1_wo", "wo1")
            identity = P["consts"].tile([128, 128], BF, name="identity")
            nc.sync.dma_start(identity, io["ident128"][:, :])
            P["identity"] = identity
            xq_res = sbuf.tile([128, 4, D], DT, tag="xq_res", bufs=1,
                               name="xq_res")
            nc.scalar.dma_start(xq_res,
                                io["x_q"].rearrange("(o p) d -> p o d", p=128))
            # w2 preload into xT's storage (free after att1). Its DMA waits on
            # xT's last reader, so it sits LAST on the scalar queue before the
            # FFN w1c stream (which isn't needed until long after).
            w2_all = sbuf.tile([128, 4, S], BF, tag="xT", bufs=1,
                               name="w2_all")
            w2_ap = io["ffn_w2"].rearrange("(b a p) n -> p a b n", a=4, p=128)
            for b in range(4):
                nc.sync.dma_start(w2_all[:, :, 512 * b:512 * b + 512],
                                  w2_ap[:, :, b, :])
            P["kt_store"] = {}

            # ---------------- Q1 projection + V1 ----------------
            qT1 = sbuf.tile([128, 4, TOK], BF, tag="qT1", bufs=1, name="qT1")
            _proj_T(nc, P, wq1_sb, xqT_sb, qT1, "q1")
            v1_sb = sbuf.tile([128, NB, H, 65], BF, tag="v1", bufs=1,
                              name="v1_sb")
            nc.vector.memset(v1_sb[:, :, :, 64], 1.0)
            for ko in range(NB):
                _v_chunk(nc, P, xT_sb, wv1_sb, v1_sb, ko)

            # ---------------- mha1 (causal self-attention) ----------------
            # v2 production is issued inside att1's pair loop: independent PE
            # work that runs while ScalarE drains att1's softmax exps.
            v2_sb = sbuf.tile([128, NB, H, 65], BF, tag="v2", bufs=1,
                              name="v2_sb")
            nc.vector.memset(v2_sb[:, :, :, 64], 1.0)

            def v2_work(q):
                return [(lambda ko: (lambda: _v_chunk(
                    nc, P, encT_sb, wv2_sb, v2_sb, ko)))(ko)
                    for ko in range(4 * q, 4 * q + 4)]

            out1 = sbuf.tile([128, 4, D], DT, tag="out1", bufs=1, name="out1")
            out1b = sbuf.tile([128, 4, D], BF, tag="out1b", bufs=1,
                              name="out1b")
            att1_fillers = [v2_work(q) for q in range(4)]
            # att1 pair 3 additionally pre-builds att2 pair 0's K^T
            att1_fillers[3] = att1_fillers[3] + _kprod_chunks(
                nc, P, wk2_sb, encT_sb, 0, ("k2", 0))
            # K2 pairs 1-2 fill att1's finalization tail
            att1_tail = (_kprod_chunks(nc, P, wk2_sb, encT_sb, 1, ("k2", 1))
                         + _kprod_chunks(nc, P, wk2_sb, encT_sb, 2, ("k2", 2)))
            _attention(nc, P, qT1, xT_sb, wk1_sb, v1_sb, True, wo1_sb,
                       xq_res, out1, out1b, "k1", att1_fillers, att1_tail)

            # ---------------- out1^T, Q2 ----------------
            out1T = sbuf.tile([128, 4, TOK], BF, tag="out1T", bufs=1,
                              name="out1T")
            # K2 pair 3 fills the transpose's PE idle (att2 would otherwise
            # produce it inside its exp-bound pair-2 loop)
            k2p3 = _kprod_chunks(nc, P, wk2_sb, encT_sb, 3, ("k2", 3))
            _transpose_q(nc, P, out1b, out1T, fillers=k2p3)
            qT2 = sbuf.tile([128, 4, TOK], BF, tag="qT2", bufs=1, name="qT2")
            _proj_T(nc, P, wq2_sb, out1T, qT2, "q2")

            # ---------------- mha2 (cross-attention, no mask) --------------
            out2 = sbuf.tile([128, 4, D], DT, tag="out2", bufs=1, name="out2")
            out2b = sbuf.tile([128, 4, D], BF, tag="out2b", bufs=1,
                              name="out2b")
            _attention(nc, P, qT2, encT_sb, wk2_sb, v2_sb, False, wo2_sb,
                       out1, out2, out2b, "k2", [[], [], [], []])

            # ---------------- FFN ----------------
            out2T = sbuf.tile([128, 4, TOK], BF, tag="out2T", bufs=1,
                              name="out2T")
            _transpose_q(nc, P, out2b, out2T)
            # h1 = relu(out2 @ w1): all 16 chunks persist (aliases encT)
            h1_all = sbuf.tile([128, 4, S], BF, tag="encT", bufs=1,
                               name="h1_all")
            for c in range(DFF // 128):
                w1c = wsb.tile([128, 4, 128], BF, tag="w1c", bufs=4,
                               name="w1c")
                eng = nc.sync if c % 2 == 0 else nc.scalar
                eng.dma_start(
                    w1c, io["ffn_w1"][:, 128 * c:128 * c + 128]
                    .rearrange("(s p) n -> p s n", p=128))
                ph = P["psum"].tile([128, 512], DT, tag="pk", bufs=2,
                                    name="ph")
                for s in range(4):
                    nc.tensor.matmul(ph, w1c[:, s, :], out2T[:, s, :],
                                     start=(s == 0), stop=(s == 3))
                nc.scalar.activation(
                    h1_all[:, c % 4, 512 * (c // 4):512 * (c // 4) + 512], ph,
                    mybir.ActivationFunctionType.Relu)
            # y = h1 @ w2, t-major so each LN overlaps the next t's matmuls
            py0 = P["psum"].tile([128, 2, 512], DT, tag="st", bufs=2,
                                 name="py0")
            py1 = P["psum"].tile([128, 2, 512], DT, tag="st", bufs=2,
                                 name="py1")
            py = [py0, py1]
            for t in range(4):
                pyt = py[t // 2][:, t % 2, :]
                for c in range(DFF // 128):
                    a, b = c % 4, c // 4
                    nc.tensor.matmul(
                        pyt,
                        h1_all[:, a, 512 * b + 128 * t:512 * b + 128 * t + 128],
                        w2_all[:, a, 512 * b:512 * b + 512],
                        start=(c == 0), stop=(c == DFF // 128 - 1),
                        skip_group_check=True)
                ya = sbuf.tile([128, 512], DT, tag="ya", bufs=2, name="ya")
                _ln_tile(nc, P, ya, pyt, out2[:, t, :])
                nc.sync.dma_start(
                    io["y"].rearrange("(o p) d -> p o d", p=128)[:, t, :], ya)
    nc.compile()
    return nc


_CACHED = None


def _get_program():
    global _CACHED
    if _CACHED is None:
        _CACHED = build_program()
    return _CACHED


def make_in_maps(inputs):
    x = np.ascontiguousarray(np.asarray(inputs["x"], FP))
    enc = np.ascontiguousarray(np.asarray(inputs["enc_output"], FP))
    lam = np.asarray(inputs["look_ahead_mask"], FP)
    pad = np.asarray(inputs["padding_mask"], FP)
    assert np.array_equal(lam[0, 0], np.triu(np.ones((S, S), FP), k=1)), \
        "kernel specialized for causal look_ahead_mask"
    assert not pad.any(), "kernel specialized for zero padding_mask"
    for p in ("mha1", "mha2"):
        for nm in ("q", "k", "v", "o"):
            assert not np.asarray(inputs[f"{p}_b{nm}"], FP).any()
    assert not np.asarray(inputs["ffn_b1"], FP).any()
    assert not np.asarray(inputs["ffn_b2"], FP).any()
    for i in (1, 2, 3):
        assert np.all(np.asarray(inputs[f"ln{i}_g"], FP) == 1.0)
        assert not np.asarray(inputs[f"ln{i}_b"], FP).any()

    weights = {w: np.ascontiguousarray(np.asarray(inputs[w], FP).astype(BFNP))
               for w in WNAMES}
    xT = [np.ascontiguousarray(x[b].T.astype(BFNP)) for b in range(B)]
    encT = [np.ascontiguousarray(enc[b].T.astype(BFNP)) for b in range(B)]
    in_maps = []
    for c in range(NCORES):
        b, r = c // G, c % G
        kp = np.arange(128)[:, None]
        u = np.arange(32)[None, :]
        M = (kp <= 4 * u + r).astype(BFNP)
        m = {
            "ident128": np.eye(128, dtype=BFNP),
            "ind8": np.kron(np.eye(8, dtype=FP), np.ones((1, 64), FP)),
            "xT": xT[b],
            "encT": encT[b],
            "xqT": np.ascontiguousarray(xT[b][:, r::4]),
            "x_q": np.ascontiguousarray(x[b, r::4, :]),
            "diagmask": M,
        }
        m.update(weights)
        in_maps.append(m)
    return in_maps


def kernel(**inputs):
    from concourse.bass_utils import run_bass_kernel_spmd
    nc = _get_program()
    in_maps = make_in_maps(inputs)
    res = run_bass_kernel_spmd(nc, in_maps, list(range(NCORES)))
    out = np.empty((B, S, D), FP)
    for c in range(NCORES):
        b, r = c // G, c % G
        out[b, r::4, :] = res.results[c]["y"]
    return out
